# revision 6
# baseline (speedup 1.0000x reference)
"""Cross-attention with 3D RoPE on 8 Trainium2 NeuronCores.

Sharding: batch*heads across cores. Core i handles batch b=i//4 and heads
(p, p+4) with p=i%4. Per core: q/k/v projections row-sharded over its 2 heads,
attention fully local per head, out-projection column-sharded; the partial
[2048, 768] outputs are summed per batch on the host (sum-gather).

Layout tricks:
- All matmuls run as float32r (fp32 data, ~2e-4 rounding, 4x the fp32 rate).
- Activations X are fed transposed (host-side) so the contraction dim is on
  partitions; q/k are produced directly transposed [d, L] for the S^T matmul.
- q/k head dims are permuted+padded to 128 rows: x1 dims in [0:48), x2 in
  [64:112) (zeros elsewhere, via zero-padded weights). RoPE then needs only
  32-aligned partition slices, which the DVE requires.
- S is computed transposed [k, q]; softmax denominators come for free from a
  ones-column appended to v in the P^T @ v_ones matmul (row 96 of the PV psum).
- No max-subtraction in softmax: |S*scale| stays < ~10, exp is safe in fp32.
"""
import sys

sys.path.insert(0, "/opt/trn_rl_repo")

import numpy as np

B, L, DIM, HEADS, HD = 2, 2048, 768, 8, 96
HDP = 128          # padded head dim for q/k
NC_ = 8            # cores
ROPE_BASE = 10000.0
SCALE = float(HD) ** -0.5
NCHUNK = DIM // 128   # 6 contraction chunks
NLT = L // 512        # 4 free-dim tiles of 512
NKT = L // 128        # 16 k tiles of 128

_nc_cache = {}


def _perm_pad_rows():
    """padded row -> original head-dim index, and the valid-row mask."""
    rows = np.full(HDP, -1, np.int64)
    for r in range(48):
        rows[r] = (r // 16) * 32 + r % 16          # x1 dims
    for r in range(48):
        rows[64 + r] = (r // 16) * 32 + 16 + r % 16  # x2 dims
    return rows


def _freq_mats():
    inv = 1.0 / (ROPE_BASE ** (np.arange(16, dtype=np.float64) / 16.0))
    fc = np.zeros((3, HDP), np.float32)
    fs = np.zeros((3, HDP), np.float32)
    for r in range(48):
        a, j = r // 16, r % 16
        fc[a, r] = inv[j]
        fc[a, 64 + r] = inv[j]
        fs[a, r] = -inv[j]
        fs[a, 64 + r] = inv[j]
    return fc, fs


def _build_program():
    import concourse.bacc as bacc
    import concourse.mybir as mybir
    from concourse import tile

    F32 = mybir.dt.float32
    F32R = mybir.dt.float32r
    AF = mybir.ActivationFunctionType

    nc = bacc.Bacc("TRN2", num_devices=NC_)

    # ---- DRAM I/O ----
    xt_q = nc.dram_tensor("xt_q", [DIM, L], F32R, kind="ExternalInput")
    xt_k = nc.dram_tensor("xt_k", [DIM, L], F32R, kind="ExternalInput")
    xt_v = nc.dram_tensor("xt_v", [DIM, L], F32R, kind="ExternalInput")
    wqk = nc.dram_tensor("wqk", [NCHUNK, 128, 4 * HDP], F32R, kind="ExternalInput")
    wv = nc.dram_tensor("wv", [NCHUNK, 128, 256], F32R, kind="ExternalInput")
    wo = nc.dram_tensor("wo", [2, HD, DIM], F32R, kind="ExternalInput")
    # pre-wrapped angle args in [-pi, pi]: cos folded as sin(ang + pi/2)
    ang_in = {
        (tag, kind): nc.dram_tensor(f"a{kind}{tag}", [HDP, L], F32, kind="ExternalInput")
        for tag in ("q", "k")
        for kind in ("c", "sx")
    }
    ones96 = nc.dram_tensor("ones96", [1, HD], F32R, kind="ExternalInput")
    onescol = nc.dram_tensor("onescol", [128, NKT], F32R, kind="ExternalInput")
    out_p = nc.dram_tensor("out_p", [L, DIM], F32, kind="ExternalOutput")

    with tile.TileContext(nc) as tc:
        from contextlib import ExitStack

        ctx = ExitStack()
        with ctx:
            sb_w = ctx.enter_context(tc.tile_pool(name="sb_w", bufs=1))
            sb_cs = ctx.enter_context(tc.tile_pool(name="sb_cs", bufs=1))
            sb_rot = ctx.enter_context(tc.tile_pool(name="sb_rot", bufs=1))
            sb_xt = ctx.enter_context(tc.tile_pool(name="sb_xt", bufs=8))
            sb_v = ctx.enter_context(tc.tile_pool(name="sb_v", bufs=1))
            sb_sc = ctx.enter_context(tc.tile_pool(name="sb_sc", bufs=2))
            sb_pt = ctx.enter_context(tc.tile_pool(name="sb_pt", bufs=3))
            sb_o = ctx.enter_context(tc.tile_pool(name="sb_o", bufs=2))
            sb_ot = ctx.enter_context(tc.tile_pool(name="sb_ot", bufs=1))
            sb_den = ctx.enter_context(tc.tile_pool(name="sb_den", bufs=2))
            sb_out = ctx.enter_context(tc.tile_pool(name="sb_out", bufs=2))
            ps_big = ctx.enter_context(
                tc.tile_pool(name="ps_big", bufs=2, space="PSUM")
            )
            ps_acc = ctx.enter_context(
                tc.tile_pool(name="ps_acc", bufs=4, space="PSUM")
            )

            # ---- constants / weights ----
            wqk_t = sb_w.tile([128, NCHUNK * 4 * HDP], F32R, name="wqk_t")
            for c in range(NCHUNK):
                nc.sync.dma_start(
                    wqk_t[:, c * 4 * HDP : (c + 1) * 4 * HDP], wqk[c]
                )
            wv_t = sb_w.tile([128, NCHUNK * 256], F32R, name="wv_t")
            for c in range(NCHUNK):
                nc.sync.dma_start(wv_t[:, c * 256 : (c + 1) * 256], wv[c])
            wo_t = [sb_w.tile([HD, DIM], F32R, name=f"wo_t{h}") for h in range(2)]
            for h in range(2):
                nc.sync.dma_start(wo_t[h][:], wo[h])
            ones96_t = sb_w.tile([1, HD], F32R, name="ones96_t")
            nc.sync.dma_start(ones96_t[:], ones96[:])

            # ---- phase A: sin of pre-wrapped angles -> C2/SX tiles ----
            # C2 = cos(ang) on both halves (bias folded on host); SX = [-sin; +sin]
            cs = {}
            with tc.tile_pool(name="sb_ang", bufs=2) as sb_ang:
                for tag in ("q", "k"):
                    c2 = sb_cs.tile([HDP, L], F32, name=f"c2_{tag}")
                    sx = sb_cs.tile([HDP, L], F32, name=f"sx_{tag}")
                    cs[tag] = (c2, sx)
                    for kind, dst in (("c", c2), ("sx", sx)):
                        for half in range(2):
                            hs = slice(half * 1024, (half + 1) * 1024)
                            at = sb_ang.tile(
                                [HDP, 1024], F32, name=f"at_{tag}{kind}{half}", tag="ang"
                            )
                            nc.sync.dma_start(at[:], ang_in[(tag, kind)][:, hs])
                            nc.scalar.activation(dst[:, hs], at[:], AF.Sin)

            # ---- phase B: projections + RoPE (q/k), v ----
            rot = {}  # (tag, h) -> [128, L] f32r rotated q/k (transposed)
            for h in range(2):
                for tag in ("q", "k"):
                    rot[(tag, h)] = sb_rot.tile(
                        [HDP, L], F32R, name=f"rot_{tag}{h}"
                    )
            xt_dram = {"q": xt_q, "k": xt_k}
            for tag in ("q", "k"):
                c2, sx = cs[tag]
                base = 0 if tag == "q" else 2 * HDP
                for lt in range(NLT):
                    sl = slice(lt * 512, (lt + 1) * 512)
                    pp = [
                        ps_acc.tile([HDP, 512], F32, name=f"pp_{tag}{h}_{lt}", tag="acc")
                        for h in range(2)
                    ]
                    for c in range(NCHUNK):
                        piece = sb_xt.tile(
                            [128, 512], F32R, name=f"xp_{tag}_{lt}_{c}", tag="xt"
                        )
                        nc.sync.dma_start(
                            piece[:], xt_dram[tag][c * 128 : (c + 1) * 128, sl]
                        )
                        for h in range(2):
                            wsl = slice(
                                c * 4 * HDP + base + h * HDP,
                                c * 4 * HDP + base + (h + 1) * HDP,
                            )
                            nc.tensor.matmul(
                                pp[h][:],
                                wqk_t[:, wsl],
                                piece[:],
                                start=(c == 0),
                                stop=(c == NCHUNK - 1),
                            )
                    for h in range(2):
                        tmp = sb_sc.tile([HDP, 512], F32, name=f"tmp_{tag}{h}_{lt}", tag="tmp")
                        xc = sb_sc.tile([HDP, 512], F32, name=f"xc_{tag}{h}_{lt}", tag="xc")
                        nc.vector.tensor_mul(tmp[0:64, :], pp[h][64:128, :], sx[0:64, sl])
                        nc.vector.tensor_mul(tmp[64:128, :], pp[h][0:64, :], sx[64:128, sl])
                        nc.vector.tensor_mul(xc[:], pp[h][:], c2[:, sl])
                        nc.vector.tensor_add(rot[(tag, h)][:, sl], xc[:], tmp[:])

            # v projection: natural [k, d] with ones column -> [128, 16*97]
            v_t = [
                sb_v.tile([128, NKT * (HD + 1)], F32R, name=f"v_t{h}")
                for h in range(2)
            ]
            for h in range(2):
                nc.sync.dma_start(
                    v_t[h].rearrange("p (k c) -> p k c", c=HD + 1)[:, :, HD],
                    onescol[:],
                )
            for ltv in range(NLT):
                pieces = []
                for c in range(NCHUNK):
                    piece = sb_xt.tile(
                        [128, 512], F32R, name=f"xv_{ltv}_{c}", tag="xt"
                    )
                    nc.sync.dma_start(
                        piece[:],
                        xt_v[c * 128 : (c + 1) * 128, ltv * 512 : (ltv + 1) * 512],
                    )
                    pieces.append(piece)
                for k4 in range(4):
                    kt = ltv * 4 + k4
                    pv = ps_acc.tile([128, 256], F32, name=f"pv_{kt}", tag="acc")
                    for c in range(NCHUNK):
                        nc.tensor.matmul(
                            pv[:],
                            pieces[c][:, k4 * 128 : (k4 + 1) * 128],
                            wv_t[:, c * 256 : (c + 1) * 256],
                            start=(c == 0),
                            stop=(c == NCHUNK - 1),
                        )
                    for h in range(2):
                        nc.vector.tensor_copy(
                            v_t[h][:, kt * (HD + 1) : kt * (HD + 1) + HD],
                            pv[:, h * HD : (h + 1) * HD],
                        )

            # ---- phase C: attention per head ----
            ot = [sb_ot.tile([HD, L], F32R, name=f"ot{h}") for h in range(2)]
            for h in range(2):
                qt_, kt_ = rot[("q", h)], rot[("k", h)]
                po = [
                    ps_acc.tile([HD + 1, 512], F32, name=f"po_{h}_{qt}", tag="acc")
                    for qt in range(NLT)
                ]
                for kc in range(NKT):
                    ksl = slice(kc * 128, (kc + 1) * 128)
                    pt = sb_pt.tile([128, L], F32R, name=f"pt_{h}_{kc}", tag="pt")
                    for half in range(2):
                        qsl = slice(half * 1024, (half + 1) * 1024)
                        st = ps_big.tile(
                            [128, 1024], F32, name=f"st_{h}_{kc}_{half}", tag="big"
                        )
                        for q2 in range(2):
                            q2sl = slice(q2 * 512, (q2 + 1) * 512)
                            nc.tensor.matmul(
                                st[:, q2sl],
                                kt_[:, ksl],
                                qt_[:, half * 1024 + q2 * 512 : half * 1024 + (q2 + 1) * 512],
                                start=True,
                                stop=True,
                            )
                        nc.scalar.activation(
                            pt[:, qsl], st[:], AF.Exp, scale=SCALE
                        )
                    for qt in range(NLT):
                        nc.tensor.matmul(
                            po[qt][:],
                            v_t[h][:, kc * (HD + 1) : (kc + 1) * (HD + 1)],
                            pt[:, qt * 512 : (qt + 1) * 512],
                            start=(kc == 0),
                            stop=(kc == NKT - 1),
                        )
                # normalize: ot = po[0:96] * (1/po[96]) broadcast over partitions
                for qt in range(NLT):
                    qsl = slice(qt * 512, (qt + 1) * 512)
                    oun = sb_o.tile([HD + 1, 512], F32, name=f"oun_{h}_{qt}", tag="oun")
                    nc.vector.tensor_copy(oun[:], po[qt][:])
                    den = sb_den.tile([1, 512], F32, name=f"den_{h}_{qt}", tag="den")
                    nc.vector.tensor_copy(den[:], oun[96:97, :])
                    rec = sb_den.tile([1, 512], F32, name=f"rec_{h}_{qt}", tag="rec")
                    nc.vector.reciprocal_approx_fast(rec[:], den[:])
                    recr = sb_den.tile([1, 512], F32R, name=f"recr_{h}_{qt}", tag="recr")
                    nc.vector.tensor_copy(recr[:], rec[:])
                    bc = ps_big.tile([HD, 512], F32, name=f"bc_{h}_{qt}", tag="big")
                    nc.tensor.matmul(
                        bc[:], ones96_t[:], recr[:], start=True, stop=True
                    )
                    nc.vector.tensor_mul(ot[h][:, qsl], oun[0:96, :], bc[:])

            # ---- phase D: out-projection, both heads accumulated ----
            for lt2 in range(NKT):
                lsl = slice(lt2 * 128, (lt2 + 1) * 128)
                pout = ps_big.tile([128, DIM], F32, name=f"pout_{lt2}", tag="big")
                for nsl in (slice(0, 512), slice(512, DIM)):
                    for h in range(2):
                        nc.tensor.matmul(
                            pout[:, nsl],
                            ot[h][:, lsl],
                            wo_t[h][:, nsl],
                            start=(h == 0),
                            stop=(h == 1),
                        )
                ost = sb_out.tile([128, DIM], F32, name=f"ost_{lt2}", tag="ost")
                nc.vector.tensor_copy(ost[:], pout[:])
                nc.sync.dma_start(out_p[lsl, :], ost[:])

    nc.compile()
    return nc


def _get_program():
    if "nc" not in _nc_cache:
        _nc_cache["nc"] = _build_program()
    return _nc_cache["nc"]


def _wrap_pi(x):
    return (x - 2.0 * np.pi * np.round(x / (2.0 * np.pi))).astype(np.float32)


def _angle_tensors(coords):
    """coords [L, 3] -> (AC, ASX) [128, L], wrapped to [-pi, pi]."""
    inv = 1.0 / (ROPE_BASE ** (np.arange(16, dtype=np.float64) / 16.0))
    ang = (coords[:, :, None].astype(np.float64) * inv).transpose(1, 2, 0)  # [3,16,L]
    ang = ang.reshape(48, -1)  # row a*16+j
    ac = np.zeros((HDP, ang.shape[1]), np.float32)
    asx = np.zeros((HDP, ang.shape[1]), np.float32)
    ac[0:48] = _wrap_pi(ang + np.pi / 2)
    ac[64:112] = ac[0:48]
    asx[0:48] = _wrap_pi(-ang)
    asx[64:112] = _wrap_pi(ang)
    return ac, asx


def _host_prep(Q_in, K_in, V_in, coords_q, coords_k, Wq, Wk, Wv, Wo):
    rows = _perm_pad_rows()
    valid = rows >= 0

    def pad_qk(W, h):
        # [768, 128] transposed, permuted+padded head rows
        Wh = W[h * HD : (h + 1) * HD, :]            # [96, 768]
        P = np.zeros((HDP, DIM), np.float32)
        P[valid] = Wh[rows[valid]]
        return np.ascontiguousarray(P.T)

    xt = {}
    for b in range(B):
        xt[("q", b)] = np.ascontiguousarray(Q_in[b].T)
        xt[("k", b)] = np.ascontiguousarray(K_in[b].T)
        xt[("v", b)] = np.ascontiguousarray(V_in[b].T)
    ang = {}
    for b in range(B):
        ang[("q", b)] = _angle_tensors(coords_q[b])
        ang[("k", b)] = _angle_tensors(coords_k[b])

    ones96 = np.ones((1, HD), np.float32)
    onescol = np.ones((128, NKT), np.float32)

    in_maps = []
    for core in range(NC_):
        b, p = core // 4, core % 4
        heads = (p, p + 4)
        WQK = np.zeros((NCHUNK, 128, 4 * HDP), np.float32)
        WV = np.zeros((NCHUNK, 128, 256), np.float32)
        WO = np.zeros((2, HD, DIM), np.float32)
        for hi, h in enumerate(heads):
            wqt = pad_qk(Wq, h)   # [768, 128]
            wkt = pad_qk(Wk, h)
            wvt = np.ascontiguousarray(Wv[h * HD : (h + 1) * HD, :].T)  # [768, 96]
            for c in range(NCHUNK):
                cs_ = slice(c * 128, (c + 1) * 128)
                WQK[c, :, hi * HDP : (hi + 1) * HDP] = wqt[cs_]
                WQK[c, :, 2 * HDP + hi * HDP : 2 * HDP + (hi + 1) * HDP] = wkt[cs_]
                WV[c, :, hi * HD : (hi + 1) * HD] = wvt[cs_]
            WO[hi] = Wo[:, h * HD : (h + 1) * HD].T
        in_maps.append(
            {
                "xt_q": xt[("q", b)],
                "xt_k": xt[("k", b)],
                "xt_v": xt[("v", b)],
                "wqk": WQK,
                "wv": WV,
                "wo": WO,
                "acq": ang[("q", b)][0],
                "asxq": ang[("q", b)][1],
                "ack": ang[("k", b)][0],
                "asxk": ang[("k", b)][1],
                "ones96": ones96,
                "onescol": onescol,
            }
        )
    return in_maps


def kernel(Q_in, K_in, V_in, coords_q, coords_k, Wq, Wk, Wv, Wo, _trace=False):
    from concourse.bass_utils import run_bass_kernel_spmd

    args = [np.asarray(a, np.float32) for a in
            (Q_in, K_in, V_in, coords_q, coords_k, Wq, Wk, Wv, Wo)]
    nc = _get_program()
    in_maps = _host_prep(*args)
    res = run_bass_kernel_spmd(
        nc, in_maps, core_ids=list(range(NC_)), trace=_trace
    )
    out = np.zeros((B, L, DIM), np.float32)
    for core in range(NC_):
        out[core // 4] += res.results[core]["out_p"]
    if _trace:
        kernel._last_results = res
    return out
